# revision 14
# baseline (speedup 1.0000x reference)
"""Trainium2 Bass kernel for the AdaGeo GNN message-passing module.

Strategy: shard target nodes (N2=2048 rows) across 8 NeuronCores (256 rows
each); landmarks [4096, *] and all weights are replicated.  Each core runs a
fully independent graph (no collectives).

Per-core layout decisions:
  - Landmark features kept in natural chunks lmf[jc] = [128, 259] bf16
    ([lm_X | lm_Y | 1]).
  - lm_X.T materialized once via 64 PE transposes -> lm_XT bf16 [128, 4096] x2.
  - Attention logits computed transposed: S.T[j, i] = lm_X @ W where
    W = ak_w.T @ ((q + aq_b)/TEMP).T  (k-side bias drops out of softmax).
  - softmax denominators via ones-vector matmuls; delta = softmax(S) used in
    place of expm1(softmax(S)) (error O(p^2), ~1e-4 of the signal).
  - attr @ lm_feature = delta.T-matmuls + colsum(lm_feature) broadcast-add;
    deg = 4098 + rou0.
  - attn2 computed unnormalized with a ones column in v2; divide at the end.
Matmul operands are bf16 (fp32 PSUM accumulation); elementwise math fp32.
"""

import os

import numpy as np

import concourse.bass as bass
import concourse.tile as tile
from concourse import bacc, mybir
from concourse.bass_utils import run_bass_kernel_spmd
from concourse.masks import make_identity

N1 = 4096
N2 = 2048
PD = 256
DZ = 128
PD2 = PD + 2          # 258
FD = PD + 2 * PD2     # 772
TEMP = float(DZ) ** 0.5
EPS = 1e-12
NCORES = 8
R = N2 // NCORES      # 256 target rows per core
NJC = N1 // 128       # 32 landmark chunks
F32 = mybir.dt.float32
BF16 = mybir.dt.bfloat16
FP = mybir.dt.float32  # elementwise dtype
MM = BF16              # matmul operand dtype
AF_T = mybir.ActivationFunctionType


def _ceil_div(a, b):
    return (a + b - 1) // b


def _chunks(total, size=128):
    """[(offset, rows), ...] covering `total` in blocks of <=size."""
    out = []
    o = 0
    while o < total:
        out.append((o, min(size, total - o)))
        o += size
    return out


STAGE = int(os.environ.get("BASS_STAGE", "6"))


def build_graph():
    nc = bacc.Bacc(None, target_bir_lowering=False)

    # ---------------- DRAM parameters ----------------
    lm_X = nc.declare_dram_parameter("lm_X", [N1, PD], F32, isOutput=False)
    lm_Y = nc.declare_dram_parameter("lm_Y", [N1, 2], F32, isOutput=False)
    lm_delay = nc.declare_dram_parameter("lm_delay", [N1], F32, isOutput=False)
    tg_X = nc.declare_dram_parameter("tg_X", [R, PD], F32, isOutput=False)
    tg_delay = nc.declare_dram_parameter("tg_delay", [R], F32, isOutput=False)
    aq_w = nc.declare_dram_parameter("aq_w", [DZ, PD], F32, isOutput=False)
    aq_b = nc.declare_dram_parameter("aq_b", [DZ], F32, isOutput=False)
    ak_w = nc.declare_dram_parameter("ak_w", [DZ, PD], F32, isOutput=False)
    w1_w = nc.declare_dram_parameter("w1_w", [PD2, PD2], F32, isOutput=False)
    w1_b = nc.declare_dram_parameter("w1_b", [PD2], F32, isOutput=False)
    w2_w = nc.declare_dram_parameter("w2_w", [PD2, PD2], F32, isOutput=False)
    w2_b = nc.declare_dram_parameter("w2_b", [PD2], F32, isOutput=False)
    pq_w = nc.declare_dram_parameter("pq_w", [DZ, FD], F32, isOutput=False)
    pq_b = nc.declare_dram_parameter("pq_b", [DZ], F32, isOutput=False)
    pk_w = nc.declare_dram_parameter("pk_w", [DZ, PD], F32, isOutput=False)
    pv_w = nc.declare_dram_parameter("pv_w", [2, 2], F32, isOutput=False)
    pv_b = nc.declare_dram_parameter("pv_b", [2], F32, isOutput=False)
    gamma1 = nc.declare_dram_parameter("gamma1", [1, 1], F32, isOutput=False)
    gamma2 = nc.declare_dram_parameter("gamma2", [1, 1], F32, isOutput=False)
    gamma3 = nc.declare_dram_parameter("gamma3", [1, 1], F32, isOutput=False)
    alpha = nc.declare_dram_parameter("alpha", [1, 1], F32, isOutput=False)
    beta = nc.declare_dram_parameter("beta", [1, 1], F32, isOutput=False)
    f_out = nc.declare_dram_parameter("f_out", [R, FD], F32, isOutput=True)
    y_out = nc.declare_dram_parameter("y_out", [R, 2], F32, isOutput=True)

    with tile.TileContext(nc) as tc:
        _emit(nc, tc, locals())
    nc.compile()
    return nc


def _emit(nc, tc, P):
    from contextlib import ExitStack

    ctx = ExitStack()
    with ctx:
        singles = ctx.enter_context(tc.tile_pool(name="singles", bufs=1))
        big = ctx.enter_context(tc.tile_pool(name="big", bufs=1))
        work = ctx.enter_context(tc.tile_pool(name="work", bufs=3))
        psum = ctx.enter_context(tc.tile_pool(name="psum", bufs=2, space="PSUM"))
        psum_st = ctx.enter_context(tc.tile_pool(name="psum_st", bufs=2, space="PSUM"))
        psum_acc = ctx.enter_context(tc.tile_pool(name="psum_acc", bufs=2, space="PSUM"))

        v = nc.vector
        s = nc.scalar
        t = nc.tensor
        g = nc.gpsimd

        # ---------------- constants ----------------
        ident_b = singles.tile([128, 128], MM)
        make_identity(nc, ident_b[:, :])
        ident_f = singles.tile([128, 128], FP)
        make_identity(nc, ident_f[:, :])
        ones_col = singles.tile([128, 1], MM)
        g.memset(ones_col[:, :], 1.0)
        ones_row = singles.tile([1, 128], MM)
        g.memset(ones_row[:, :], 1.0)
        ones_row_f = singles.tile([1, 128], FP)
        g.memset(ones_row_f[:, :], 1.0)

        def peT(out_psum, in_sb):
            """PE transpose: out[F, P] = in_[P, F].T (psum dest)."""
            p = in_sb.partition_size()
            ident = ident_b if in_sb.dtype == MM else ident_f
            t.transpose(out_psum, in_sb, ident[:p, :p])


        def _finish_stub():
            stub = singles.tile([128, FD], FP, name="stub")
            g.memset(stub[:, :], 0.0)
            for ic in range(2):
                g.dma_start(out=P["f_out"][ic * 128 : (ic + 1) * 128, :],
                            in_=stub[:, :])
                g.dma_start(out=P["y_out"][ic * 128 : (ic + 1) * 128, :],
                            in_=stub[:, 0:2])
        # ---------------- phase 0: scalars ----------------
        # scal row slots: 0..4 = alpha,beta,g1,g2,g3 ; 8..13 = pvw00,01,10,11,pvb0,pvb1
        # computed slots: 16..21 = m1,c1,m2,c2,m3,c3  (m_k=-g_k*alpha, c_k=-g_k*beta)
        scal = singles.tile([1, 32], FP)
        g.memset(scal[:, :], 0.0)
        g.dma_start(out=scal[0:1, 0:1], in_=P["alpha"][:, :])
        g.dma_start(out=scal[0:1, 1:2], in_=P["beta"][:, :])
        g.dma_start(out=scal[0:1, 2:3], in_=P["gamma1"][:, :])
        g.dma_start(out=scal[0:1, 3:4], in_=P["gamma2"][:, :])
        g.dma_start(out=scal[0:1, 4:5], in_=P["gamma3"][:, :])
        g.dma_start(out=scal[0:1, 8:12], in_=P["pv_w"].rearrange("a b -> (a b)")[None, :])
        g.dma_start(out=scal[0:1, 12:14], in_=P["pv_b"][None, :])
        for k in range(3):  # m_k, c_k for gamma1..3
            gk = scal[0:1, 2 + k : 3 + k]
            v.tensor_scalar(out=scal[0:1, 16 + 2 * k : 17 + 2 * k], in0=gk,
                            scalar1=scal[0:1, 0:1], scalar2=-1.0,
                            op0=mybir.AluOpType.mult, op1=mybir.AluOpType.mult)
            v.tensor_scalar(out=scal[0:1, 17 + 2 * k : 18 + 2 * k], in0=gk,
                            scalar1=scal[0:1, 1:2], scalar2=-1.0,
                            op0=mybir.AluOpType.mult, op1=mybir.AluOpType.mult)
        # broadcast all 32 scalar slots to 128 partitions (exact fp32 matmul)
        ps = psum.tile([128, 32], FP, tag="mm")
        t.matmul(ps[:, :], ones_row_f[0:1, :], scal[0:1, :], start=True, stop=True)
        scal_bc = singles.tile([128, 32], FP)
        v.tensor_copy(scal_bc[:, :], ps[:, :])

        def sc(idx):  # [128,1] broadcast scalar AP
            return scal_bc[:, idx : idx + 1]

        # ---------------- phase 0b: delays ----------------
        # lm delay -> dso [128, jc, {ds, 1}] bf16 (matvec lhsT chunks)
        ld = work.tile([128, NJC], FP, tag="ld")
        g.dma_start(out=ld[:, :], in_=P["lm_delay"].rearrange("(c p) -> p c", p=128))
        dso = singles.tile([128, NJC, 2], MM)
        g.memset(dso[:, :, :], 1.0)
        s.activation(dso[:, :, 0], ld[:, :], AF_T.Exp, bias=sc(17), scale=sc(16))

        td = work.tile([128, 2], FP, tag="ld")
        g.dma_start(out=td[:, :], in_=P["tg_delay"].rearrange("(c p) -> p c", p=128))
        rou0_c = singles.tile([128, 2], FP)
        s.activation(rou0_c[:, :], td[:, :], AF_T.Exp, bias=sc(19), scale=sc(18))
        rou1_c = singles.tile([128, 2], FP)
        s.activation(rou1_c[:, :], td[:, :], AF_T.Exp, bias=sc(21), scale=sc(20))
        inv0_c = singles.tile([128, 2], FP)
        v.tensor_scalar_add(inv0_c[:, :], rou0_c[:, :], float(N1 + 2) + EPS)
        v.reciprocal(inv0_c[:, :], inv0_c[:, :])
        inv1_c = singles.tile([128, 2], FP)
        v.tensor_scalar_add(inv1_c[:, :], rou1_c[:, :], 1.0 + EPS)
        v.reciprocal(inv1_c[:, :], inv1_c[:, :])
        # rou0 as bf16 row [1, 256] for the outer-product matmul
        psr = psum.tile([2, 256], FP, tag="mm")
        for ic in range(2):
            peT(psr[0:1, ic * 128 : (ic + 1) * 128], rou0_c[:, ic : ic + 1])
        rou0_row = singles.tile([1, 256], MM)
        v.tensor_copy(rou0_row[0:1, :], psr[0:1, :])

        if STAGE < 2:
            _finish_stub()
            return
        # ---------------- phase 1: target-side projections ----------------
        tgX_nat = singles.tile([128, 2, PD], FP)  # [p, ic, c]
        g.dma_start(out=tgX_nat[:, :, :], in_=P["tg_X"].rearrange("(i p) c -> p i c", p=128))
        # tgXT[cc] = [128, 256] bf16 : tg_X.T chunks (i on free)
        tgXT = singles.tile([128, 2, 256], MM)
        for cc in range(2):
            pt = psum.tile([128, 256], MM, tag="mm")
            for ic in range(2):
                xin = work.tile([128, 128], MM, tag="cast")
                v.tensor_copy(xin[:, :], tgX_nat[:, ic, cc * 128 : (cc + 1) * 128])
                peT(pt[:, ic * 128 : (ic + 1) * 128], xin[:, :])
            v.tensor_copy(tgXT[:, cc, :], pt[:, :])

        # weights for attn1
        akw = singles.tile([128, PD], MM)
        g.dma_start(out=akw[:, :], in_=P["ak_w"][:, :])
        aqw = work.tile([128, PD], MM, tag="wload")
        g.dma_start(out=aqw[:, :], in_=P["aq_w"][:, :])
        aqb_col = singles.tile([128, 1], FP)
        g.dma_start(out=aqb_col[:, 0], in_=P["aq_b"][:])
        aqwT = singles.tile([128, 2, 128], MM)
        for cc in range(2):
            pt = psum.tile([128, 128], MM, tag="mm")
            peT(pt[:, :], aqw[:, cc * 128 : (cc + 1) * 128])
            v.tensor_copy(aqwT[:, cc, :], pt[:, :])

        # qsT = ((tg_X @ aq_w.T + aq_b)/TEMP).T  -> [128(dz), 256(i)]
        pq = psum.tile([128, 256], FP, tag="mm")
        for cc in range(2):
            t.matmul(pq[:, :], aqwT[:, cc, :], tgXT[:, cc, :], start=(cc == 0), stop=(cc == 1))
        qsT = singles.tile([128, 256], MM)
        v.tensor_scalar(out=qsT[:, :], in0=pq[:, :], scalar1=aqb_col[:, :],
                        scalar2=1.0 / TEMP, op0=mybir.AluOpType.add,
                        op1=mybir.AluOpType.mult)

        # W1[cc] = (ak_w.T @ qsT) chunks [128(c), 256(i)]
        W1 = singles.tile([128, 2, 256], MM)
        for cc in range(2):
            pw = psum.tile([128, 256], FP, tag="mm")
            t.matmul(pw[:, :], akw[:, cc * 128 : (cc + 1) * 128], qsT[:, :],
                     start=True, stop=True)
            v.tensor_copy(W1[:, cc, :], pw[:, :])

        if STAGE < 3:
            _finish_stub()
            return
        # ---------------- phase 2: landmark pipeline ----------------
        lmf = big.tile([128, NJC, PD2 + 1], MM)       # [lm_X | lm_Y | 1] bf16
        g.dma_start(out=lmf[:, :, 0:PD],
                    in_=P["lm_X"].rearrange("(c p) m -> p c m", p=128))
        g.dma_start(out=lmf[:, :, PD:PD2],
                    in_=P["lm_Y"].rearrange("(c p) m -> p c m", p=128))
        g.memset(lmf[:, :, PD2 : PD2 + 1], 1.0)

        lm_XT = [big.tile([128, N1], MM, tag=f"lmxt{cc}", name=f"lm_XT{cc}") for cc in range(2)]
        TGRP = 4  # jc per transpose-psum batch
        for grp in range(NJC // TGRP):
            for cc in range(2):
                pt = psum_st.tile([128, TGRP * 128], MM, tag="pt")
                for k in range(TGRP):
                    jc = grp * TGRP + k
                    peT(pt[:, k * 128 : (k + 1) * 128],
                        lmf[:, jc, cc * 128 : (cc + 1) * 128])
                v.tensor_copy(lm_XT[cc][:, grp * TGRP * 128 : (grp + 1) * TGRP * 128],
                              pt[:, :])

        if STAGE < 4:
            _finish_stub()
            return
        # accumulators: RV rows 0..1 = [ds|1].T @ lmf ; Z = colsum(E)
        RVt = psum_acc.tile([2, 512], FP, tag="acc")
        Zt = psum_acc.tile([1, 512], FP, tag="acc")
        RV = RVt[0:2, 0 : PD2 + 1]
        Z = Zt[0:1, 0:R]

        ET = big.tile([128, NJC, R], MM)  # E.T then delta (in place)
        SGRP = 2
        for grp in range(NJC // SGRP):
            st = psum_st.tile([128, SGRP * R], FP, tag="st")
            for k in range(SGRP):
                jc = grp * SGRP + k
                for cc in range(2):
                    t.matmul(st[:, k * R : (k + 1) * R],
                             lm_XT[cc][:, jc * 128 : (jc + 1) * 128],
                             W1[:, cc, :], start=(cc == 0), stop=(cc == 1))
            s.activation(ET[:, grp * SGRP : (grp + 1) * SGRP, :].rearrange("p a b -> p (a b)"),
                         st[:, :], AF_T.Exp)
            for k in range(SGRP):
                jc = grp * SGRP + k
                t.matmul(Z, ones_col[:, :], ET[:, jc, :],
                         start=(jc == 0), stop=(jc == NJC - 1))
        for jc in range(NJC):
            t.matmul(RV, dso[:, jc, :], lmf[:, jc, :],
                     start=(jc == 0), stop=(jc == NJC - 1))

        if STAGE < 5:
            _finish_stub()
            return
        # ---------------- phase 3: softmax, attrF, tg chain ----------------
        # delta = E / Z : broadcast 1/Z along partitions via K=1 matmul
        iZf = work.tile([1, R], FP, tag="vecrowf")
        v.reciprocal(iZf[0:1, :], Z)
        iZ = work.tile([1, R], MM, tag="vecrow")
        v.tensor_copy(iZ[0:1, :], iZf[0:1, :])
        pb = psum.tile([128, R], FP, tag="mm")
        t.matmul(pb[:, :], ones_row[0:1, :], iZ[0:1, :], start=True, stop=True)
        INVb = singles.tile([128, R], MM)
        v.tensor_copy(INVb[:, :], pb[:, :])
        for jc in range(NJC):
            v.tensor_tensor(out=ET[:, jc, :], in0=ET[:, jc, :], in1=INVb[:, :],
                            op=mybir.AluOpType.mult)

        # landmark-side router values.  Row 1 of RV cannot be sliced directly
        # (partition offsets must be 0/32/64/96), so transpose RV into column
        # form [f, {ds@lmf, colsum}] and work per-partition.
        fch = _chunks(PD2)  # [(0,128),(128,128),(256,2)]
        RVs = singles.tile([2, PD2 + 1], FP)
        v.tensor_copy(RVs[:, :], RV)
        RVc = [singles.tile([r_, 2], FP, tag=f"RVc{i}", name=f"RVc{i}")
               for i, (o_, r_) in enumerate(fch)]
        for i, (o_, r_) in enumerate(fch):
            prv = psum.tile([r_, 2], FP, tag="mm")
            peT(prv[:, :], RVs[:, o_ : o_ + r_])
            v.tensor_copy(RVc[i][:, :], prv[:, :])
        # rden_bc = 1/(1 + sum_ds + EPS) broadcast to all partitions
        rdem = work.tile([1, 1], FP, tag="vecrow")
        v.tensor_scalar_add(rdem[0:1, :], RVs[0:1, PD2 : PD2 + 1], 1.0 + EPS)
        v.reciprocal(rdem[0:1, :], rdem[0:1, :])
        prd = psum.tile([128, 1], FP, tag="mm")
        t.matmul(prd[:, :], ones_row_f[0:1, :], rdem[0:1, :], start=True, stop=True)
        rden_bc = singles.tile([128, 1], FP)
        v.tensor_copy(rden_bc[:, :], prd[:, :])
        # rp0_col = (ds@lmf + colsum/N1) * rden ; bf16 copy for matmul lhsT
        rp0c_b = [work.tile([r_, 1], MM, tag=f"rpcb{i}", name=f"rp0cb{i}")
                  for i, (o_, r_) in enumerate(fch)]
        for i, (o_, r_) in enumerate(fch):
            rc = work.tile([r_, 1], FP, tag=f"rc{i}", name=f"rc{i}")
            v.tensor_scalar(out=rc[:, :], in0=RVc[i][:, 1:2], scalar1=1.0 / N1,
                            scalar2=None, op0=mybir.AluOpType.mult)
            v.tensor_tensor(out=rc[:, :], in0=rc[:, :], in1=RVc[i][:, 0:1],
                            op=mybir.AluOpType.add)
            v.tensor_scalar_mul(rc[:, :], rc[:, :], rden_bc[0 : r_, :])
            v.tensor_copy(rp0c_b[i][:, :], rc[:, :])
        # cs / r0e rows rebuilt from columns via tiny transposes
        pcs = psum.tile([1, PD2], FP, tag="mm")
        for i, (o_, r_) in enumerate(fch):
            peT(pcs[0:1, o_ : o_ + r_], RVc[i][:, 1:2])
        cs_row = singles.tile([1, PD2], FP)
        v.tensor_copy(cs_row[0:1, :], pcs[0:1, :])
        cs_b = singles.tile([1, PD2], MM)
        v.tensor_copy(cs_b[0:1, :], cs_row[0:1, :])
        r0e_b = singles.tile([1, PD2], MM)
        v.tensor_scalar_mul(r0e_b[0:1, :], cs_row[0:1, :], 1.0 / N1)

        # attrF + assembled numerator -> p0 (natural, bf16) ; also fp32 copy
        p0_nat = singles.tile([128, 2, PD2], MM)
        for ic in range(2):
            pa = psum.tile([128, PD2], FP, tag="mm")
            for jc in range(NJC):
                t.matmul(pa[:, :], ET[:, jc, ic * 128 : (ic + 1) * 128],
                         lmf[:, jc, 0:PD2], start=(jc == 0), stop=False)
            t.matmul(pa[:, :], ones_row[0:1, :], cs_b[0:1, :], start=False, stop=False)
            t.matmul(pa[:, :], rou0_row[0:1, ic * 128 : (ic + 1) * 128],
                     r0e_b[0:1, :], start=False, stop=True)
            tmp = work.tile([128, PD], FP, tag="num")
            v.tensor_tensor(out=tmp[:, :], in0=pa[:, 0:PD], in1=tgX_nat[:, ic, :],
                            op=mybir.AluOpType.add)
            v.tensor_scalar_mul(p0_nat[:, ic, 0:PD], tmp[:, :], inv0_c[:, ic : ic + 1])
            v.tensor_scalar_mul(p0_nat[:, ic, PD:PD2], pa[:, PD:PD2],
                                inv0_c[:, ic : ic + 1])

        # transpose p0 -> p0T chunks [(128|128|2) x 256]
        p0T = [singles.tile([r_, 256], MM, tag=f"p0T{i}", name=f"p0T{i}") for i, (o_, r_) in enumerate(fch)]
        for i, (o_, r_) in enumerate(fch):
            pt = psum.tile([r_, 256], MM, tag="mm")
            for ic in range(2):
                peT(pt[:, ic * 128 : (ic + 1) * 128], p0_nat[:, ic, o_ : o_ + r_])
            v.tensor_copy(p0T[i][:, :], pt[:, :])

        # w1/w2 transposed weights (bf16) + bias rows
        def load_wT(wname, bname):
            w_nat = [work.tile([r_, PD2], MM, tag=f"wn{r_}", name=f"{wname}n{o_}") for (o_, r_) in fch]
            for i, (o_, r_) in enumerate(fch):
                g.dma_start(out=w_nat[i][:, :], in_=P[wname][o_ : o_ + r_, :])
            wT = [singles.tile([r_, PD2], MM, tag=f"{wname}T{i}", name=f"{wname}T{i}")
                  for i, (o_, r_) in enumerate(fch)]
            for i, (o_, r_) in enumerate(fch):      # wT[i] rows = w cols o_..o_+r_
                pt = psum.tile([r_, PD2], MM, tag="mm")
                for j2, (o2, r2) in enumerate(fch):
                    peT(pt[:, o2 : o2 + r2], w_nat[j2][:, o_ : o_ + r_])
                v.tensor_copy(wT[i][:, :], pt[:, :])
            brow = singles.tile([1, PD2], FP, tag=f"{bname}r")
            g.dma_start(out=brow[0:1, :], in_=P[bname][None, :])
            pbb = psum.tile([128, PD2], FP, tag="mm")
            t.matmul(pbb[:, :], ones_row_f[0:1, :], brow[0:1, :], start=True, stop=True)
            b_bc = singles.tile([128, PD2], FP, tag=f"{bname}bc")
            v.tensor_copy(b_bc[:, :], pbb[:, :])
            return wT, brow, b_bc

        w1T, w1b_row, w1b_bc = load_wT("w1_w", "w1_b")
        w2T, w2b_row, w2b_bc = load_wT("w2_w", "w2_b")

        # router_1 = rp0 @ w1_w.T + w1_b  (rp0 already in column form)
        pr1 = psum.tile([1, PD2], FP, tag="mm")
        for i, (o_, r_) in enumerate(fch):
            t.matmul(pr1[:, :], rp0c_b[i][:, :], w1T[i][:, :],
                     start=(i == 0), stop=(i == 2))
        r1_row = singles.tile([1, PD2], FP)
        v.tensor_tensor(out=r1_row[0:1, :], in0=pr1[0:1, :], in1=w1b_row[0:1, :],
                        op=mybir.AluOpType.add)
        pr1b = psum.tile([128, PD2], FP, tag="mm")
        t.matmul(pr1b[:, :], ones_row_f[0:1, :], r1_row[0:1, :], start=True, stop=True)
        r1_bc = singles.tile([128, PD2], FP)
        v.tensor_copy(r1_bc[:, :], pr1b[:, :])

        # tg1 = p0 @ w1_w.T + w1_b (natural fp32) ; tg1T via PE transpose
        def layer(pT, wT, b_bc, out_tag):
            nat = singles.tile([128, 2, PD2], FP, tag=f"{out_tag}nat")
            for ic in range(2):
                pn = psum.tile([128, PD2], FP, tag="mm")
                for i, (o_, r_) in enumerate(fch):
                    t.matmul(pn[:, :], pT[i][:, ic * 128 : (ic + 1) * 128],
                             wT[i][:, :], start=(i == 0), stop=(i == 2))
                v.tensor_tensor(out=nat[:, ic, :], in0=pn[:, :], in1=b_bc[:, :],
                                op=mybir.AluOpType.add)
            return nat

        tg1_nat = layer(p0T, w1T, w1b_bc, "tg1")

        # p1 = (tg1 + rou1*r1_bc) * inv1  (bf16) -> p1T
        p1_nat = singles.tile([128, 2, PD2], MM)
        for ic in range(2):
            v.scalar_tensor_tensor(out=p1_nat[:, ic, :], in0=r1_bc[:, :],
                                   scalar=rou1_c[:, ic : ic + 1], in1=tg1_nat[:, ic, :],
                                   op0=mybir.AluOpType.mult, op1=mybir.AluOpType.add)
            v.tensor_scalar_mul(p1_nat[:, ic, :], p1_nat[:, ic, :],
                                inv1_c[:, ic : ic + 1])
        p1T = [singles.tile([r_, 256], MM, tag=f"p1T{i}", name=f"p1T{i}") for i, (o_, r_) in enumerate(fch)]
        for i, (o_, r_) in enumerate(fch):
            pt = psum.tile([r_, 256], MM, tag="mm")
            for ic in range(2):
                peT(pt[:, ic * 128 : (ic + 1) * 128], p1_nat[:, ic, o_ : o_ + r_])
            v.tensor_copy(p1T[i][:, :], pt[:, :])

        tg2_nat = layer(p1T, w2T, w2b_bc, "tg2")
        tg2T = [singles.tile([r_, 256], MM, tag=f"tg2T{i}", name=f"tg2T{i}") for i, (o_, r_) in enumerate(fch)]
        tg1T = [singles.tile([r_, 256], MM, tag=f"tg1T{i}", name=f"tg1T{i}") for i, (o_, r_) in enumerate(fch)]
        for src_nat, dstT in ((tg1_nat, tg1T), (tg2_nat, tg2T)):
            for i, (o_, r_) in enumerate(fch):
                pt = psum.tile([r_, 256], FP, tag="mm")
                for ic in range(2):
                    peT(pt[:, ic * 128 : (ic + 1) * 128], src_nat[:, ic, o_ : o_ + r_])
                v.tensor_copy(dstT[i][:, :], pt[:, :])

        # final_tg_feature out: [tg_X | tg1 | tg2]
        g.dma_start(out=P["f_out"][:, 0:PD], in_=P["tg_X"][:, :])
        for ic in range(2):
            g.dma_start(out=P["f_out"][ic * 128 : (ic + 1) * 128, PD : PD + PD2],
                        in_=tg1_nat[:, ic, :])
            g.dma_start(out=P["f_out"][ic * 128 : (ic + 1) * 128, PD + PD2 : FD],
                        in_=tg2_nat[:, ic, :])

        if STAGE < 6:
            stub = singles.tile([128, 2], FP, name="ystub")
            g.memset(stub[:, :], 0.0)
            for ic in range(2):
                g.dma_start(out=P["y_out"][ic * 128 : (ic + 1) * 128, :],
                            in_=stub[:, :])
            return
        # ---------------- phase 4: attn2 ----------------
        # q2sT = ((final @ pq_w.T + pq_b)/TEMP).T via ragged finalT chunks
        pqw = work.tile([128, FD], MM, tag="pqw")
        g.dma_start(out=pqw[:, :], in_=P["pq_w"][:, :])
        pqb_col = singles.tile([128, 1], FP)
        g.dma_start(out=pqb_col[:, 0], in_=P["pq_b"][:])
        finalT = [tgXT[:, 0, :], tgXT[:, 1, :]] + [x[:, :] for x in tg1T] + \
                 [x[:, :] for x in tg2T]
        f_offsets = [(0, 128), (128, 128)] + \
                    [(PD + o_, r_) for (o_, r_) in fch] + \
                    [(PD + PD2 + o_, r_) for (o_, r_) in fch]
        pq2 = psum.tile([128, 256], FP, tag="mm")
        for i, (o_, r_) in enumerate(f_offsets):
            ptw = psum.tile([r_, 128], MM, tag="mm")
            peT(ptw[:, :], pqw[:, o_ : o_ + r_])
            pqwT_i = work.tile([r_, 128], MM, tag=f"pqwT{r_}")
            v.tensor_copy(pqwT_i[:, :], ptw[:, :])
            t.matmul(pq2[:, :], pqwT_i[:, :], finalT[i],
                     start=(i == 0), stop=(i == len(f_offsets) - 1))
        q2sT = singles.tile([128, 256], MM)
        v.tensor_scalar(out=q2sT[:, :], in0=pq2[:, :], scalar1=pqb_col[:, :],
                        scalar2=1.0 / TEMP, op0=mybir.AluOpType.add,
                        op1=mybir.AluOpType.mult)

        pkw = singles.tile([128, PD], MM)
        g.dma_start(out=pkw[:, :], in_=P["pk_w"][:, :])
        W2 = singles.tile([128, 2, 256], MM)
        for cc in range(2):
            pw = psum.tile([128, 256], FP, tag="mm")
            t.matmul(pw[:, :], pkw[:, cc * 128 : (cc + 1) * 128], q2sT[:, :],
                     start=True, stop=True)
            v.tensor_copy(W2[:, cc, :], pw[:, :])

        # v2e chunks [128, jc, 3] bf16 : [v2 | 1]
        lmY_c = work.tile([128, NJC, 2], FP, tag="lmy")
        g.dma_start(out=lmY_c[:, :, :], in_=P["lm_Y"].rearrange("(c p) m -> p c m", p=128))
        v2e = singles.tile([128, NJC, 3], MM)
        g.memset(v2e[:, :, :], 1.0)
        t0 = work.tile([128, NJC], FP, tag="v2t")
        for cix in range(2):  # output column of v2
            w_a, w_b, b_ = 8 + 2 * cix, 9 + 2 * cix, 12 + cix
            v.tensor_scalar(out=t0[:, :], in0=lmY_c[:, :, 0], scalar1=sc(w_a),
                            scalar2=sc(b_), op0=mybir.AluOpType.mult,
                            op1=mybir.AluOpType.add)
            v.scalar_tensor_tensor(out=v2e[:, :, cix], in0=lmY_c[:, :, 1],
                                   scalar=sc(w_b), in1=t0[:, :],
                                   op0=mybir.AluOpType.mult, op1=mybir.AluOpType.add)

        # S2.T -> exp -> E2T ; ZT = v2e.T @ E2T accumulate
        E2T = big.tile([128, NJC, R], MM, tag="e2t")
        ZT = psum_acc.tile([3, 512], FP, tag="acc")
        for grp in range(NJC // SGRP):
            st = psum_st.tile([128, SGRP * R], FP, tag="st")
            for k in range(SGRP):
                jc = grp * SGRP + k
                for cc in range(2):
                    t.matmul(st[:, k * R : (k + 1) * R],
                             lm_XT[cc][:, jc * 128 : (jc + 1) * 128],
                             W2[:, cc, :], start=(cc == 0), stop=(cc == 1))
            s.activation(E2T[:, grp * SGRP : (grp + 1) * SGRP, :].rearrange("p a b -> p (a b)"),
                         st[:, :], AF_T.Exp)
            for k in range(SGRP):
                jc = grp * SGRP + k
                t.matmul(ZT[:, 0:R], v2e[:, jc, :], E2T[:, jc, :],
                         start=(jc == 0), stop=(jc == NJC - 1))

        # y: transpose ZT -> [128, 3] per i-chunk, divide, DMA natural rows
        ZTs = singles.tile([3, R], FP)
        v.tensor_copy(ZTs[:, :], ZT[:, 0:R])
        y_nat = singles.tile([128, 2, 2], FP)
        for ic in range(2):
            pz = psum.tile([128, 3], FP, tag="mm")
            peT(pz[:, :], ZTs[:, ic * 128 : (ic + 1) * 128])
            zc = work.tile([128, 3], FP, tag="zc")
            v.tensor_copy(zc[:, :], pz[:, :])
            zi = work.tile([128, 1], FP, tag="zi")
            v.reciprocal(zi[:, :], zc[:, 2:3])
            v.tensor_scalar_mul(y_nat[:, ic, :], zc[:, 0:2], zi[:, :])
            g.dma_start(out=P["y_out"][ic * 128 : (ic + 1) * 128, :],
                        in_=y_nat[:, ic, :])


_CACHE = {}


def _get_graph():
    if "nc" not in _CACHE:
        _CACHE["nc"] = build_graph()
    return _CACHE["nc"]


def build_in_maps(inputs):
    inputs = {k: np.ascontiguousarray(np.asarray(v, dtype=np.float32))
              for k, v in inputs.items()}
    in_maps = []
    for c in range(NCORES):
        sl = slice(c * R, (c + 1) * R)
        m = {
            "lm_X": inputs["lm_X"], "lm_Y": inputs["lm_Y"],
            "lm_delay": inputs["lm_delay"],
            "tg_X": np.ascontiguousarray(inputs["tg_X"][sl]),
            "tg_delay": np.ascontiguousarray(inputs["tg_delay"][sl]),
            "aq_w": inputs["aq_w"], "aq_b": inputs["aq_b"], "ak_w": inputs["ak_w"],
            "w1_w": inputs["w1_w"], "w1_b": inputs["w1_b"],
            "w2_w": inputs["w2_w"], "w2_b": inputs["w2_b"],
            "pq_w": inputs["pq_w"], "pq_b": inputs["pq_b"], "pk_w": inputs["pk_w"],
            "pv_w": inputs["pv_w"], "pv_b": inputs["pv_b"],
            "gamma1": inputs["gamma1"], "gamma2": inputs["gamma2"],
            "gamma3": inputs["gamma3"], "alpha": inputs["alpha"],
            "beta": inputs["beta"],
        }
        in_maps.append(m)
    return in_maps


def kernel(**inputs):
    nc = _get_graph()
    in_maps = build_in_maps(inputs)
    res = run_bass_kernel_spmd(nc, in_maps, core_ids=list(range(NCORES))).results
    y = np.concatenate([res[c]["y_out"] for c in range(NCORES)], axis=0)
    f = np.concatenate([res[c]["f_out"] for c in range(NCORES)], axis=0)
    return y.astype(np.float32), f.astype(np.float32)


if __name__ == "__main__":
    nc = build_graph()
    print("graph built ok")
